# revision 32
# baseline (speedup 1.0000x reference)
"""Trainium2 Bass kernel for nn_MultiHeadAttention (B=8192, D=1024, 16 heads
used only via the softmax scale 1/8).

Strategy (8 NeuronCores, row-sharded attention + AllGather collectives):
  - Rows (batch axis) of the attention output are sharded: core c owns rows
    [c*1024, (c+1)*1024).
  - Algebraic restructuring removes the K projection and all Q/K weight
    transposes:
        E[i, j] = Q_i . K_j = (Wk^T Q_i) . x_j + (Q_i . bk)
    The per-row constant Q_i.bk cancels in softmax, so with
        M' = Wq^T Wk          (from natural-layout weights, no transposes)
        Z^T = M'^T x^T + (Wk^T bq)  (per-core, local rows only)
    the energy is E^T[j, i] = sum_d x^T[d, j] * Z^T[d, i].
  - Each core transposes only its local 1024 rows of x AND Wv on the PE
    (XBAR DMA-transposes are avoided entirely: they serialize against
    collectives on the shared TOPSP hardware). Transpose PSUM drains are
    batched 4-per-tile so DVE copies never pace the PE.
  - The full x^T (fp16) and full V (bf16) are assembled with AllGather
    collectives, each split in row-halves. Phase 2 processes key/value
    blocks ROTATED per core (partition_id + dynamic-offset DMA): each
    core starts with its OWN rows straight from SBUF, then all h=0
    remote halves, then all h=1 halves -- so only the first x/V gathers
    are timing-critical and the cross-core launch-skew barrier (43-125us
    observed) hides under local compute.
  - Attention runs in the transposed-energy ("E^T") layout so no probability
    transpose is needed:
        P^T = exp(E^T * 0.125)           (no max subtraction; |logit| small)
        out_unnorm[i, o] = sum_j P^T[j, i] * V'[j, o]   (V' = V + bv)
        s[i] = sum_j P^T[j, i]   (ones-matmul, ONE psum tile accumulates
                                  across all 16 j-blocks -- no DVE RMW)
        out = out_unnorm / s     (bv needs no epilogue term: softmax rows
                                  sum to 1, so attn @ (V+bv) = attn@V + bv)
  - e-phases run eagerly (EAGER_E deep) ahead of o-phases so the V gather
    hides behind energy matmuls even with a late barrier; V-row prefetches
    ride the otherwise-idle GPSIMD queue.
  - The last 3 j-blocks merge into one finale whose attn@V shares a single
    PSUM accumulation, leaving the DVE idle for the fused
    normalize(ACT per-partition scale)+store writeback -- no serial tail.
  - M' runs fp32r (full rate, measured) with an oo-outer accumulation (8
    live PSUM banks) so its matmuls stream behind the chunk-interleaved
    Wq/Wk DMA arrivals; everything else is fp16 (P/V bf16 because exp can
    exceed fp16 range). Sustained PE clock is 13/16-throttled (1.95 GHz);
    phase 2 runs at that streaming roofline (~262 ns per N=512 matmul).
"""

import sys

sys.path.insert(0, "/opt/trn_rl_repo")

import numpy as np

import concourse.bass as bass  # noqa: F401
import concourse.tile as tile
from concourse import bacc, mybir
from concourse.bass_utils import run_bass_kernel_spmd
from concourse.masks import make_identity

B = 8192
D = 1024
P = 128
NCORES = 8
R = B // NCORES  # 1024 rows per core
JBLK = 512  # j-block (keys/values) streamed per iteration
NJB = B // JBLK  # 16
DO = D // P  # 8 feature chunks of 128
IC = R // P  # 8 row chunks of 128 per core
F32 = mybir.dt.float32
F32R = mybir.dt.float32r
BF16 = mybir.dt.bfloat16
F16 = mybir.dt.float16
AF = mybir.ActivationFunctionType
ALU = mybir.AluOpType
SCALE = 0.125  # 1/sqrt(head_dim=64)
EAGER_E = 5  # remote e-phases run before the first remote o-phase


def build_program():
    nc = bacc.Bacc(
        "TRN2", target_bir_lowering=False, debug=False, num_devices=NCORES
    )
    x_loc = nc.dram_tensor("x_loc", [R, D], F32, kind="ExternalInput").ap()
    w_q = nc.dram_tensor("Wq", [D, D], F32, kind="ExternalInput").ap()
    w_k = nc.dram_tensor("Wk", [D, D], F32, kind="ExternalInput").ap()
    w_v = nc.dram_tensor("Wv", [D, D], F32, kind="ExternalInput").ap()
    b_q = nc.dram_tensor("bq", [D], F32, kind="ExternalInput").ap()
    b_v = nc.dram_tensor("bv", [D], F32, kind="ExternalInput").ap()
    out_loc = nc.dram_tensor("out_loc", [R, D], F32, kind="ExternalOutput").ap()

    with tile.TileContext(nc) as tc:
        _body(nc, tc, x_loc, w_q, w_k, w_v, b_q, b_v, out_loc)
    nc.compile()
    return nc


def _body(nc, tc, x_loc, w_q, w_k, w_v, b_q, b_v, out_loc):
    from contextlib import ExitStack

    outer = ExitStack()
    outer.__enter__()
    # ---- persistent pools (whole kernel) ----
    const_pool = outer.enter_context(tc.tile_pool(name="const", bufs=1))
    identity = const_pool.tile([P, P], F32)
    make_identity(nc, identity)
    ones_f32 = const_pool.tile([P, 2], F32)
    nc.vector.memset(ones_f32, 1.0)
    ones = const_pool.tile([P, 2], BF16)
    nc.vector.tensor_copy(out=ones, in_=ones_f32)
    bq_sb = const_pool.tile([P, DO], F32R)
    ones_row = const_pool.tile([1, P], F32)
    nc.vector.memset(ones_row, 1.0)
    g_row = const_pool.tile([1, D], F32)  # g = Wk^T bq as a row
    g_col = const_pool.tile([P, DO], F32)  # g in column layout (round trip)
    bv_bc = const_pool.tile([P, D], F32)

    # persistent fp16 local x^T: energy stationary for the LOCAL j-blocks,
    # Z^T moving operand, V stationary, and the AllGather payload
    xt_pool = outer.enter_context(tc.tile_pool(name="xt", bufs=1))
    xt16 = xt_pool.tile([P, DO, R], F16)  # 2 MB

    zt_pool = outer.enter_context(tc.tile_pool(name="zt", bufs=1))
    zt = zt_pool.tile([P, DO, R], F16)  # Z^T: [d_in, dd, i]  (2 MB)

    vs_pool = outer.enter_context(tc.tile_pool(name="vsb", bufs=1))
    v_sb = vs_pool.tile([P, IC, D], BF16)  # local V rows, SBUF copy (2 MB)

    sums_pool = outer.enter_context(tc.tile_pool(name="sums", bufs=1))
    rs_all = sums_pool.tile([P, 2 * IC], F32)

    # DRAM scratch: local x^T / V shards + AllGather outputs (Shared).
    # Both gathers are split in row-halves so the first half lands on the
    # collective stream (serialized behind the skew barrier) ~35us earlier.
    dram = outer.enter_context(tc.tile_pool(name="dram", bufs=1, space="DRAM"))
    xt_loc_h = [dram.tile([DO, P, JBLK], F16, name=f"xt_loc_{h}") for h in range(2)]
    xt_gh = [
        dram.tile([NCORES, DO, P, JBLK], F16, addr_space="Shared", name=f"xt_g{h}")
        for h in range(2)
    ]
    v_loc_h = [dram.tile([JBLK, D], BF16, name=f"v_loc_{h}") for h in range(2)]
    v_gh = [
        dram.tile([NCORES, JBLK, D], BF16, addr_space="Shared", name=f"v_g{h}")
        for h in range(2)
    ]
    g_d = dram.tile([D], F32)  # row->column round trip for g

    # =========================================================
    # Phase 1: x^T (local), AG(x^T); M' = Wq^T Wk; Z^T; V, AG(V)
    # =========================================================
    with ExitStack() as p1:
        # -- transpose local x rows into fp16 x^T; ship shard + AllGather --
        # deep row pool so all 8 x-row DMAs stream back-to-back: the shard
        # must reach DRAM fast, the AllGather is triggered off it
        with tc.tile_pool(name="rows", bufs=4) as row_pool, tc.tile_pool(
            name="tp_ps", bufs=3, space="PSUM"
        ) as tp_psum:
            for ic in range(IC):
                xrow = row_pool.tile([P, D], F32, tag="row")
                nc.sync.dma_start(xrow, x_loc[ic * P : (ic + 1) * P, :])
                # 4 transposes per PSUM tile, ONE batched DVE drain each:
                # unbatched drains pace PE transposes at ~550ns instead
                # of ~275ns
                for dq in range(2):
                    tp = tp_psum.tile([P, 4 * P], F32, tag="tp")
                    for k in range(4):
                        dd = 4 * dq + k
                        nc.tensor.transpose(
                            tp[:, k * P : (k + 1) * P],
                            xrow[:, dd * P : (dd + 1) * P],
                            identity,
                        )
                    nc.vector.tensor_copy(
                        out=xt16[
                            :, 4 * dq : 4 * dq + 4, ic * P : (ic + 1) * P
                        ],
                        in_=tp.rearrange("p (k q) -> p k q", k=4),
                    )
        # small bias loads AFTER the x rows so xrow0 leads the sync queue
        nc.sync.dma_start(
            bq_sb, b_q.rearrange("(oo p) -> p oo", p=P).bitcast(F32R)
        )
        nc.sync.dma_start(bv_bc[0:1, :], b_v[None, :])

        # broadcast bv across all 128 partitions with a K=1 matmul (PE is
        # idle here waiting for weight chunks)
        with tc.tile_pool(name="bv_psum", bufs=2, space="PSUM") as bvp:
            for oh in range(2):
                pt = bvp.tile([P, 512], F32, tag="bvp")
                nc.tensor.matmul(
                    pt,
                    ones_row,
                    bv_bc[0:1, oh * 512 : (oh + 1) * 512],
                    start=True,
                    stop=True,
                )
                nc.vector.tensor_copy(
                    out=bv_bc[:, oh * 512 : (oh + 1) * 512], in_=pt
                )

        # -- Wv loads on the sync queue right behind the x rows (weights
        #    own the scalar queue); transposed later on the PE -- no XBAR
        #    DMA-transposes anywhere, so collectives never serialize
        #    against them on the shared TOPSP hardware --
        wv_pool = p1.enter_context(tc.tile_pool(name="wv", bufs=1))
        wvt = wv_pool.tile([P, DO, D], F16)  # Wv^T: [d, dd, o] (2 MB)
        wvb_pool = p1.enter_context(tc.tile_pool(name="wvb", bufs=1))
        wvbig = wvb_pool.tile([P, DO, D], F32)
        # per-chunk Wv loads: the PE transposes of chunk oo unblock as
        # soon as that chunk lands instead of after the full 4 MB
        wv_r = w_v.rearrange("(oo p) d -> p oo d", p=P)
        for oo in range(DO):
            nc.scalar.dma_start(wvbig[:, oo, :], wv_r[:, oo, :])
        for h in range(2):
            for dd in range(DO):
                nc.sync.dma_start(
                    xt_loc_h[h][dd], xt16[:, dd, h * JBLK : (h + 1) * JBLK]
                )
        # AG(x^T) half 0 fires as soon as its stores land; the other three
        # collectives are emitted later in DEADLINE order (AG2a before
        # AG1b: phase 2 consumes all h=0 blocks before any h=1 block)
        nc.gpsimd.collective_compute(
            "AllGather",
            mybir.AluOpType.bypass,
            replica_groups=[list(range(NCORES))],
            ins=[xt_loc_h[0].opt()],
            outs=[xt_gh[0].opt()],
        )

        # -- M' = Wq^T Wk from natural-layout weights, oo-OUTER so the
        #    matmuls stream right behind the chunked weight DMAs --
        mp_pool = p1.enter_context(tc.tile_pool(name="mp", bufs=1))
        mp16 = mp_pool.tile([P, DO, D], F16)  # M'[d', dp, d] (2 MB)
        with ExitStack() as wqk:
            wq_pool = wqk.enter_context(tc.tile_pool(name="wq", bufs=1))
            wq_sb = wq_pool.tile([P, DO, D], F32R)  # Wq rows: [o, oo, d']
            wk_pool = wqk.enter_context(tc.tile_pool(name="wk", bufs=1))
            wk_sb = wk_pool.tile([P, DO, D], F32R)  # Wk rows: [o, oo, d]
            wq_r = w_q.rearrange("(oo p) d -> p oo d", p=P).bitcast(F32R)
            wk_r = w_k.rearrange("(oo p) d -> p oo d", p=P).bitcast(F32R)
            # chunk-interleaved loads: (wq_oo, wk_oo) pairs land every ~3us
            for oo in range(DO):
                nc.scalar.dma_start(wq_sb[:, oo, :], wq_r[:, oo, :])
                nc.scalar.dma_start(wk_sb[:, oo, :], wk_r[:, oo, :])
            with tc.tile_pool(name="mm8", bufs=8, space="PSUM") as mm8:
                for dh in range(2):
                    pm = [
                        mm8.tile([P, 512], F32, tag="mm", name=f"pm{dp}")
                        for dp in range(DO)
                    ]
                    for oo in range(DO):
                        for dp in range(DO):
                            nc.tensor.matmul(
                                pm[dp],
                                wq_sb[:, oo, dp * P : (dp + 1) * P],
                                wk_sb[:, oo, dh * 512 : (dh + 1) * 512],
                                start=(oo == 0),
                                stop=(oo == DO - 1),
                            )
                    for dp in range(DO):
                        nc.vector.tensor_copy(
                            out=mp16[:, dp, dh * 512 : (dh + 1) * 512],
                            in_=pm[dp],
                        )
                    # g = Wk^T bq between the sweeps: wk is fully resident
                    # by now and the round trip finishes well before the
                    # first Z^T drain needs g_col
                    pg = mm8.tile([1, 512], F32, tag="mm", name="pg")
                    for oo in range(DO):
                        nc.tensor.matmul(
                            pg,
                            bq_sb[:, oo : oo + 1],
                            wk_sb[:, oo, dh * 512 : (dh + 1) * 512],
                            start=(oo == 0),
                            stop=(oo == DO - 1),
                        )
                    nc.vector.tensor_copy(
                        out=g_row[:, dh * 512 : (dh + 1) * 512], in_=pg
                    )
        nc.scalar.dma_start(g_d[None, :], g_row)
        nc.scalar.dma_start(g_col, g_d.rearrange("(dd p) -> p dd", p=P))

        # -- Wv^T via PE transposes (cheap: fills the tail of the weight
        #    DMA window), fp16 out straight from PSUM --
        with tc.tile_pool(name="tv_ps", bufs=3, space="PSUM") as tv_psum:
            for oo in range(DO):
                for dq in range(2):
                    tv = tv_psum.tile([P, 4 * P], F32, tag="tv")
                    for k in range(4):
                        dd = 4 * dq + k
                        nc.tensor.transpose(
                            tv[:, k * P : (k + 1) * P],
                            wvbig[:, oo, dd * P : (dd + 1) * P],
                            identity,
                        )
                    nc.vector.tensor_copy(
                        out=wvt[:, 4 * dq : 4 * dq + 4, oo * P : (oo + 1) * P],
                        in_=tv.rearrange("p (k q) -> p k q", k=4),
                    )

        # -- V FIRST (independent of Z^T): its stores release the V
        #    AllGather trigger ~35us earlier, widening the collective
        #    stream's slack against late skew barriers --
        zp = p1.enter_context(tc.tile_pool(name="z_ps", bufs=4, space="PSUM"))
        # -- V local (natural [j, o]; bias bv deferred to epilogue): kept
        #    in SBUF for the local j-blocks AND shipped to DRAM for AG(V) --
        for jj in range(IC):
            pv_h = [
                zp.tile([P, 512], F32, tag="zp", name="pv") for _ in range(2)
            ]
            for dd in range(DO):
                for oh in range(2):
                    nc.tensor.matmul(
                        pv_h[oh],
                        xt16[:, dd, jj * P : (jj + 1) * P],
                        wvt[:, dd, oh * 512 : (oh + 1) * 512],
                        start=(dd == 0),
                        stop=(dd == DO - 1),
                    )
            for oh in range(2):
                # V' = V + bv: softmax rows sum to 1, so attn @ (V + bv)
                # = attn @ V + bv -- the output bias rides the V gather
                # for free and the epilogue needs no bias stage at all
                nc.vector.tensor_tensor(
                    v_sb[:, jj, oh * 512 : (oh + 1) * 512],
                    pv_h[oh],
                    bv_bc[:, oh * 512 : (oh + 1) * 512],
                    ALU.add,
                )
            h, jr = jj // (JBLK // P), jj % (JBLK // P)
            nc.scalar.dma_start(
                v_loc_h[h][jr * P : (jr + 1) * P, :], v_sb[:, jj, :]
            )
            if jr == JBLK // P - 1:
                nc.gpsimd.collective_compute(
                    "AllGather",
                    mybir.AluOpType.bypass,
                    replica_groups=[list(range(NCORES))],
                    ins=[v_loc_h[h].opt()],
                    outs=[v_gh[h].opt()],
                )
                if h == 0:
                    # AG(x^T) half 1: consumed only by the h=1 remote
                    # blocks, which run LAST -- huge deadline slack
                    nc.gpsimd.collective_compute(
                        "AllGather",
                        mybir.AluOpType.bypass,
                        replica_groups=[list(range(NCORES))],
                        ins=[xt_loc_h[1].opt()],
                        outs=[xt_gh[1].opt()],
                    )

        # -- Z^T = M'^T x^T, bias g added on the PSUM drain --
        for dd in range(DO):
            for ih in range(R // JBLK):
                pz = zp.tile([P, JBLK], F32, tag="zp")
                for dp in range(DO):
                    nc.tensor.matmul(
                        pz,
                        mp16[:, dp, dd * P : (dd + 1) * P],
                        xt16[:, dp, ih * JBLK : (ih + 1) * JBLK],
                        start=(dp == 0),
                        stop=(dp == DO - 1),
                    )
                nc.vector.tensor_scalar_add(
                    zt[:, dd, ih * JBLK : (ih + 1) * JBLK],
                    pz,
                    g_col[:, dd : dd + 1],
                )

    # =========================================================
    # Phase 2: streamed attention in E^T layout, rank-rotated
    # =========================================================
    with ExitStack() as p2:
        oa_pool = p2.enter_context(tc.tile_pool(name="oacc", bufs=1))
        outacc = oa_pool.tile([P, IC, D], F32)  # 4 MB

        xtb_pool = p2.enter_context(tc.tile_pool(name="xtb", bufs=3))
        v_pool = p2.enter_context(tc.tile_pool(name="vtb", bufs=3))
        pt_pool = p2.enter_context(tc.tile_pool(name="ptb", bufs=6))
        fin_pool = p2.enter_context(tc.tile_pool(name="fin", bufs=3))
        e_psum = p2.enter_context(tc.tile_pool(name="e_ps", bufs=3, space="PSUM"))
        o_psum = p2.enter_context(tc.tile_pool(name="o_ps", bufs=4, space="PSUM"))
        s_psum = p2.enter_context(tc.tile_pool(name="s_ps", bufs=1, space="PSUM"))

        # rank-rotated slot order: slot 0 is OUR shard (SBUF, no DMA, no
        # dependency on either AllGather); remote slots are read with
        # dynamic-offset DMAs indexed off partition_id
        pid_sy = nc.sync.partition_id()
        rot_sy = [(pid_sy + t) % NCORES for t in range(1, NCORES)]
        pid_gp = nc.gpsimd.partition_id()
        rot_gp = [(pid_gp + t) % NCORES for t in range(1, NCORES)]

        def vtb_prefetch(th):
            """issue the V-rows DMA for remote j-block (t,h) on the GPSIMD
            queue, which is empty in phase 2 (collectives all triggered in
            phase 1): a wait on the V AllGather here never stalls the exp
            (scalar) or xtb (sync) queues feeding the energy pipeline."""
            t, h = th
            vtb = v_pool.tile([P, JBLK // P, D], BF16, tag="vtb", name="vtb")
            nc.gpsimd.dma_start(
                vtb,
                v_gh[h][rot_gp[t - 1]].rearrange("(jj p) o -> p jj o", p=P),
            )
            return vtb

        def e_phase(th):
            """energy matmuls + exp for j-block (t,h); returns ptb.
            Slot t = 0 is local (straight from SBUF)."""
            t, h = th
            if t == 0:
                xsrc = xt16[:, :, h * JBLK : (h + 1) * JBLK]
            else:
                xtb = xtb_pool.tile([P, DO, JBLK], F16, tag="xtb", name="xtb")
                nc.sync.dma_start(
                    xtb, xt_gh[h][rot_sy[t - 1]].rearrange("dd p r -> p dd r")
                )
                xsrc = xtb
            ptb = pt_pool.tile([P, JBLK // P, R], BF16, tag="ptb", name="ptb")
            for jj in range(JBLK // P):
                pe_h = [
                    e_psum.tile([P, JBLK], F32, tag="pe", name="pe")
                    for _ in range(R // JBLK)
                ]
                for dd in range(DO):
                    for ih in range(R // JBLK):
                        nc.tensor.matmul(
                            pe_h[ih],
                            xsrc[:, dd, jj * P : (jj + 1) * P],
                            zt[:, dd, ih * JBLK : (ih + 1) * JBLK],
                            start=(dd == 0),
                            stop=(dd == DO - 1),
                        )
                for ih in range(R // JBLK):
                    nc.scalar.activation(
                        ptb[:, jj, ih * JBLK : (ih + 1) * JBLK],
                        pe_h[ih],
                        AF.Exp,
                        scale=SCALE,
                    )
            return ptb

        def vsrc_of(th, vtb, jj):
            t, h = th
            if t == 0:
                return v_sb[:, h * (JBLK // P) + jj, :]
            return vtb[:, jj, :]

        # ONE psum tile accumulates the exp-sums across ALL 16 j-blocks:
        # PSUM accumulation is free (has_written bits), no per-block DVE
        # read-modify-write, no bank churn at the o-drain
        ps = s_psum.tile([P, 2 * IC], F32, tag="ps", name="ps")

        def o_phase(th, ptb, vtb, first, acc_eng=None):
            """attn@V accumulation + exp-sums for one j-block. acc_eng
            overrides the engine for the outacc accumulation (the
            penultimate block uses GPSIMD so the DVE queue is clear when
            the fused finale needs it)."""
            eng = acc_eng or nc.vector
            for ic in range(IC):
                po_h = [
                    o_psum.tile([P, 512], F32, tag="po", name="po")
                    for _ in range(2)
                ]
                for jj in range(JBLK // P):
                    vs = vsrc_of(th, vtb, jj)
                    for oh in range(2):
                        nc.tensor.matmul(
                            po_h[oh],
                            ptb[:, jj, ic * P : (ic + 1) * P],
                            vs[:, oh * 512 : (oh + 1) * 512],
                            start=(jj == 0),
                            stop=(jj == JBLK // P - 1),
                        )
                    nc.tensor.matmul(
                        ps[:, 2 * ic : 2 * ic + 2],
                        ptb[:, jj, ic * P : (ic + 1) * P],
                        ones,
                        start=(first and ic == 0 and jj == 0),
                        stop=False,
                    )
                for oh in range(2):
                    dst = outacc[:, ic, oh * 512 : (oh + 1) * 512]
                    if first:
                        nc.vector.tensor_copy(out=dst, in_=po_h[oh])
                    else:
                        eng.tensor_tensor(dst, po_h[oh], dst, ALU.add)

        def o_phase_final(parts):
            """merged LAST THREE j-blocks: their attn@V shares one PSUM
            accumulation per (ic, oh), so outacc's last writer is the
            block before them -- the DVE is fully idle when the fused
            normalize + bias + store chain needs it. Exp-sums for all
            merged blocks run first so the reciprocal is ready early."""
            nb_ = len(parts)
            for bi, (th_, ptb_, vtb_) in enumerate(parts):
                for ic in range(IC):
                    for jj in range(JBLK // P):
                        nc.tensor.matmul(
                            ps[:, 2 * ic : 2 * ic + 2],
                            ptb_[:, jj, ic * P : (ic + 1) * P],
                            ones,
                            start=False,
                            stop=(
                                bi == nb_ - 1
                                and ic == IC - 1
                                and jj == JBLK // P - 1
                            ),
                        )
            nc.vector.reciprocal(rs_all, ps)
            for ic in range(IC):
                po_h = [
                    o_psum.tile([P, 512], F32, tag="po", name="po")
                    for _ in range(2)
                ]
                for bi, (th_, ptb_, vtb_) in enumerate(parts):
                    for jj in range(JBLK // P):
                        vs = vsrc_of(th_, vtb_, jj)
                        for oh in range(2):
                            nc.tensor.matmul(
                                po_h[oh],
                                ptb_[:, jj, ic * P : (ic + 1) * P],
                                vs[:, oh * 512 : (oh + 1) * 512],
                                start=(bi == 0 and jj == 0),
                                stop=(bi == nb_ - 1 and jj == JBLK // P - 1),
                            )
                for oh in range(2):
                    # DVE (add outacc) -> ACT (1/s scale) -> DMA out
                    ofin = fin_pool.tile([P, 512], F32, tag="ofin")
                    nc.vector.tensor_tensor(
                        ofin,
                        po_h[oh],
                        outacc[:, ic, oh * 512 : (oh + 1) * 512],
                        ALU.add,
                    )
                    ofin2 = fin_pool.tile([P, 512], F32, tag="ofin2")
                    nc.scalar.activation(
                        ofin2, ofin, AF.Copy,
                        scale=rs_all[:, 2 * ic : 2 * ic + 1],
                    )
                    nc.sync.dma_start(
                        out_loc[ic * P : (ic + 1) * P, oh * 512 : (oh + 1) * 512],
                        ofin2,
                    )

        # Block sequence: the two LOCAL blocks first (they need neither
        # AllGather), then ALL h=0 remotes, then ALL h=1 remotes -- so
        # only AG1a/AG2a are timing-critical; the h=1 gathers get ~200us
        # of extra deadline slack. Emission: an eager run of EAGER_E
        # remote e-phases builds runway so the V gather hides behind
        # energy work even when the skew barrier lands late; after that
        # e/o interleave 1:1 and the o's drain at the end.
        seq = [(0, 0), (0, 1)]
        seq += [(t, 0) for t in range(1, NCORES)]
        seq += [(t, 1) for t in range(1, NCORES)]

        NFIN = 3  # blocks merged into the finale
        order = [("e", 0), ("e", 1), ("o", 0), ("o", 1)]
        order += [("e", b) for b in range(2, 2 + EAGER_E)]
        nb = 2 + EAGER_E
        for b in range(2, NJB - NFIN):
            order.append(("o", b))
            if nb < NJB:
                order.append(("e", nb))
                nb += 1
        while nb < NJB:
            order.append(("e", nb))
            nb += 1
        order.append(("fin", NJB - NFIN))

        ptbs, vtbs = {}, {}
        for kind, b in order:
            if kind == "e":
                ptbs[b] = e_phase(seq[b])
            elif kind == "o":
                # prefetch the NEXT remote o-block's V rows one full
                # o-phase (~18us) ahead of its consumption
                if 2 <= b + 1 < NJB:
                    vtbs[b + 1] = vtb_prefetch(seq[b + 1])
                o_phase(seq[b], ptbs.pop(b), vtbs.pop(b, None), b == 0)
            else:
                # remaining vtb prefetches lead their use by the upfront
                # sum-matmul batch (~5us)
                for bf in range(b + 1, NJB):
                    vtbs[bf] = vtb_prefetch(seq[bf])
                o_phase_final(
                    [(seq[bf], ptbs.pop(bf), vtbs.pop(bf, None))
                     for bf in range(b, NJB)]
                )

    outer.close()


_NC_CACHE = None


def _get_program():
    global _NC_CACHE
    if _NC_CACHE is None:
        _NC_CACHE = build_program()
    return _NC_CACHE


def _run(inputs, trace=False):
    nc = _get_program()
    x = np.ascontiguousarray(np.asarray(inputs["x"], dtype=np.float32))
    common = {
        k: np.ascontiguousarray(np.asarray(inputs[k], dtype=np.float32))
        for k in ("Wq", "Wk", "Wv", "bq", "bv")
    }
    in_maps = [
        {"x_loc": np.ascontiguousarray(x[c * R : (c + 1) * R]), **common}
        for c in range(NCORES)
    ]
    res = run_bass_kernel_spmd(
        nc, in_maps, core_ids=list(range(NCORES)), trace=trace
    )
    out = np.concatenate([res.results[c]["out_loc"] for c in range(NCORES)], axis=0)
    return out.reshape(B, D, 1).astype(np.float32), res


def kernel(**inputs):
    out, _ = _run(inputs, trace=False)
    return out


# revision 33
# speedup vs baseline: 1.0048x; 1.0048x over previous
"""Trainium2 Bass kernel for nn_MultiHeadAttention (B=8192, D=1024, 16 heads
used only via the softmax scale 1/8).

Strategy (8 NeuronCores, row-sharded attention + AllGather collectives):
  - Rows (batch axis) of the attention output are sharded: core c owns rows
    [c*1024, (c+1)*1024).
  - Algebraic restructuring removes the K projection and all Q/K weight
    transposes:
        E[i, j] = Q_i . K_j = (Wk^T Q_i) . x_j + (Q_i . bk)
    The per-row constant Q_i.bk cancels in softmax, so with
        M' = Wq^T Wk          (from natural-layout weights, no transposes)
        Z^T = M'^T x^T + (Wk^T bq)  (per-core, local rows only)
    the energy is E^T[j, i] = sum_d x^T[d, j] * Z^T[d, i].
  - Each core transposes only its local 1024 rows of x AND Wv on the PE
    (XBAR DMA-transposes are avoided entirely: they serialize against
    collectives on the shared TOPSP hardware). Transpose PSUM drains are
    batched 4-per-tile so DVE copies never pace the PE.
  - The full x^T (fp16) and full V (bf16) are assembled with AllGather
    collectives, each split in row-halves. Phase 2 processes key/value
    blocks ROTATED per core (partition_id + dynamic-offset DMA): each
    core starts with its OWN rows straight from SBUF, then all h=0
    remote halves, then all h=1 halves -- so only the first x/V gathers
    are timing-critical and the cross-core launch-skew barrier (43-125us
    observed) hides under local compute.
  - Attention runs in the transposed-energy ("E^T") layout so no probability
    transpose is needed:
        P^T = exp(E^T * 0.125)           (no max subtraction; |logit| small)
        out_unnorm[i, o] = sum_j P^T[j, i] * V'[j, o]   (V' = V + bv)
        s[i] = sum_j P^T[j, i]   (ones-matmul, ONE psum tile accumulates
                                  across all 16 j-blocks -- no DVE RMW)
        out = out_unnorm / s     (bv needs no epilogue term: softmax rows
                                  sum to 1, so attn @ (V+bv) = attn@V + bv)
  - e-phases run eagerly (EAGER_E deep) ahead of o-phases so the V gather
    hides behind energy matmuls even with a late barrier; V-row prefetches
    ride the otherwise-idle GPSIMD queue.
  - The last 3 j-blocks merge into one finale whose attn@V shares a single
    PSUM accumulation, leaving the DVE idle for the fused
    normalize(ACT per-partition scale)+store writeback -- no serial tail.
  - M' runs fp32r (full rate, measured) with an oo-outer accumulation (8
    live PSUM banks) so its matmuls stream behind the chunk-interleaved
    Wq/Wk DMA arrivals; everything else is fp16 (P/V bf16 because exp can
    exceed fp16 range). Sustained PE clock is 13/16-throttled (1.95 GHz);
    phase 2 runs at that streaming roofline (~262 ns per N=512 matmul).
"""

import sys

sys.path.insert(0, "/opt/trn_rl_repo")

import numpy as np

import concourse.bass as bass  # noqa: F401
import concourse.tile as tile
from concourse import bacc, mybir
from concourse.bass_utils import run_bass_kernel_spmd
from concourse.masks import make_identity

B = 8192
D = 1024
P = 128
NCORES = 8
R = B // NCORES  # 1024 rows per core
JBLK = 512  # j-block (keys/values) streamed per iteration
NJB = B // JBLK  # 16
DO = D // P  # 8 feature chunks of 128
IC = R // P  # 8 row chunks of 128 per core
F32 = mybir.dt.float32
F32R = mybir.dt.float32r
BF16 = mybir.dt.bfloat16
F16 = mybir.dt.float16
AF = mybir.ActivationFunctionType
ALU = mybir.AluOpType
SCALE = 0.125  # 1/sqrt(head_dim=64)
EAGER_E = 5  # remote e-phases run before the first remote o-phase


def build_program():
    nc = bacc.Bacc(
        "TRN2", target_bir_lowering=False, debug=False, num_devices=NCORES
    )
    x_loc = nc.dram_tensor("x_loc", [R, D], F32, kind="ExternalInput").ap()
    w_q = nc.dram_tensor("Wq", [D, D], F32, kind="ExternalInput").ap()
    w_k = nc.dram_tensor("Wk", [D, D], F32, kind="ExternalInput").ap()
    w_v = nc.dram_tensor("Wv", [D, D], F32, kind="ExternalInput").ap()
    b_q = nc.dram_tensor("bq", [D], F32, kind="ExternalInput").ap()
    b_v = nc.dram_tensor("bv", [D], F32, kind="ExternalInput").ap()
    out_loc = nc.dram_tensor("out_loc", [R, D], F32, kind="ExternalOutput").ap()

    with tile.TileContext(nc) as tc:
        _body(nc, tc, x_loc, w_q, w_k, w_v, b_q, b_v, out_loc)
    nc.compile()
    return nc


def _body(nc, tc, x_loc, w_q, w_k, w_v, b_q, b_v, out_loc):
    from contextlib import ExitStack

    outer = ExitStack()
    outer.__enter__()
    # ---- persistent pools (whole kernel) ----
    const_pool = outer.enter_context(tc.tile_pool(name="const", bufs=1))
    identity = const_pool.tile([P, P], F32)
    make_identity(nc, identity)
    ones_f32 = const_pool.tile([P, 2], F32)
    nc.vector.memset(ones_f32, 1.0)
    ones = const_pool.tile([P, 2], BF16)
    nc.vector.tensor_copy(out=ones, in_=ones_f32)
    bq_sb = const_pool.tile([P, DO], F32R)
    ones_row = const_pool.tile([1, P], F32)
    nc.vector.memset(ones_row, 1.0)
    g_row = const_pool.tile([1, D], F32)  # g = Wk^T bq as a row
    g_col = const_pool.tile([P, DO], F32)  # g in column layout (round trip)
    bv_bc = const_pool.tile([P, D], F32)

    # persistent fp16 local x^T: energy stationary for the LOCAL j-blocks,
    # Z^T moving operand, V stationary, and the AllGather payload
    xt_pool = outer.enter_context(tc.tile_pool(name="xt", bufs=1))
    xt16 = xt_pool.tile([P, DO, R], F16)  # 2 MB

    zt_pool = outer.enter_context(tc.tile_pool(name="zt", bufs=1))
    zt = zt_pool.tile([P, DO, R], F16)  # Z^T: [d_in, dd, i]  (2 MB)

    vs_pool = outer.enter_context(tc.tile_pool(name="vsb", bufs=1))
    v_sb = vs_pool.tile([P, IC, D], BF16)  # local V rows, SBUF copy (2 MB)

    sums_pool = outer.enter_context(tc.tile_pool(name="sums", bufs=1))
    rs_all = sums_pool.tile([P, 2 * IC], F32)

    # DRAM scratch: local x^T / V shards + AllGather outputs (Shared).
    # Both gathers are split in row-halves so the first half lands on the
    # collective stream (serialized behind the skew barrier) ~35us earlier.
    dram = outer.enter_context(tc.tile_pool(name="dram", bufs=1, space="DRAM"))
    xt_loc_h = [dram.tile([DO, P, JBLK], F16, name=f"xt_loc_{h}") for h in range(2)]
    xt_gh = [
        dram.tile([NCORES, DO, P, JBLK], F16, addr_space="Shared", name=f"xt_g{h}")
        for h in range(2)
    ]
    v_loc_h = [dram.tile([JBLK, D], BF16, name=f"v_loc_{h}") for h in range(2)]
    v_gh = [
        dram.tile([NCORES, JBLK, D], BF16, addr_space="Shared", name=f"v_g{h}")
        for h in range(2)
    ]
    g_d = dram.tile([D], F32)  # row->column round trip for g

    # =========================================================
    # Phase 1: x^T (local), AG(x^T); M' = Wq^T Wk; Z^T; V, AG(V)
    # =========================================================
    with ExitStack() as p1:
        # -- transpose local x rows into fp16 x^T; ship shard + AllGather --
        # deep row pool so all 8 x-row DMAs stream back-to-back: the shard
        # must reach DRAM fast, the AllGather is triggered off it
        with tc.tile_pool(name="rows", bufs=4) as row_pool, tc.tile_pool(
            name="tp_ps", bufs=3, space="PSUM"
        ) as tp_psum:
            for ic in range(IC):
                xrow = row_pool.tile([P, D], F32, tag="row")
                nc.sync.dma_start(xrow, x_loc[ic * P : (ic + 1) * P, :])
                # 4 transposes per PSUM tile, ONE batched DVE drain each:
                # unbatched drains pace PE transposes at ~550ns instead
                # of ~275ns
                for dq in range(2):
                    tp = tp_psum.tile([P, 4 * P], F32, tag="tp")
                    for k in range(4):
                        dd = 4 * dq + k
                        nc.tensor.transpose(
                            tp[:, k * P : (k + 1) * P],
                            xrow[:, dd * P : (dd + 1) * P],
                            identity,
                        )
                    nc.vector.tensor_copy(
                        out=xt16[
                            :, 4 * dq : 4 * dq + 4, ic * P : (ic + 1) * P
                        ],
                        in_=tp.rearrange("p (k q) -> p k q", k=4),
                    )
        # small bias loads AFTER the x rows so xrow0 leads the sync queue
        nc.sync.dma_start(
            bq_sb, b_q.rearrange("(oo p) -> p oo", p=P).bitcast(F32R)
        )
        nc.sync.dma_start(bv_bc[0:1, :], b_v[None, :])

        # broadcast bv across all 128 partitions with a K=1 matmul (PE is
        # idle here waiting for weight chunks)
        with tc.tile_pool(name="bv_psum", bufs=2, space="PSUM") as bvp:
            for oh in range(2):
                pt = bvp.tile([P, 512], F32, tag="bvp")
                nc.tensor.matmul(
                    pt,
                    ones_row,
                    bv_bc[0:1, oh * 512 : (oh + 1) * 512],
                    start=True,
                    stop=True,
                )
                nc.vector.tensor_copy(
                    out=bv_bc[:, oh * 512 : (oh + 1) * 512], in_=pt
                )

        # -- Wv loads on the sync queue right behind the x rows (weights
        #    own the scalar queue); transposed later on the PE -- no XBAR
        #    DMA-transposes anywhere, so collectives never serialize
        #    against them on the shared TOPSP hardware --
        wv_pool = p1.enter_context(tc.tile_pool(name="wv", bufs=1))
        wvt = wv_pool.tile([P, DO, D], F16)  # Wv^T: [d, dd, o] (2 MB)
        wvb_pool = p1.enter_context(tc.tile_pool(name="wvb", bufs=1))
        wvbig = wvb_pool.tile([P, DO, D], F32)
        # per-chunk Wv loads: the PE transposes of chunk oo unblock as
        # soon as that chunk lands instead of after the full 4 MB
        wv_r = w_v.rearrange("(oo p) d -> p oo d", p=P)
        for oo in range(DO):
            nc.scalar.dma_start(wvbig[:, oo, :], wv_r[:, oo, :])
        for h in range(2):
            for dd in range(DO):
                nc.sync.dma_start(
                    xt_loc_h[h][dd], xt16[:, dd, h * JBLK : (h + 1) * JBLK]
                )
        # AG(x^T) half 0 fires as soon as its stores land; the other three
        # collectives are emitted later in DEADLINE order (AG2a before
        # AG1b: phase 2 consumes all h=0 blocks before any h=1 block)
        nc.gpsimd.collective_compute(
            "AllGather",
            mybir.AluOpType.bypass,
            replica_groups=[list(range(NCORES))],
            ins=[xt_loc_h[0].opt()],
            outs=[xt_gh[0].opt()],
        )

        # -- M' = Wq^T Wk from natural-layout weights, oo-OUTER so the
        #    matmuls stream right behind the chunked weight DMAs --
        mp_pool = p1.enter_context(tc.tile_pool(name="mp", bufs=1))
        mp16 = mp_pool.tile([P, DO, D], F16)  # M'[d', dp, d] (2 MB)
        with ExitStack() as wqk:
            wq_pool = wqk.enter_context(tc.tile_pool(name="wq", bufs=1))
            wq_sb = wq_pool.tile([P, DO, D], F32R)  # Wq rows: [o, oo, d']
            wk_pool = wqk.enter_context(tc.tile_pool(name="wk", bufs=1))
            wk_sb = wk_pool.tile([P, DO, D], F32R)  # Wk rows: [o, oo, d]
            wq_r = w_q.rearrange("(oo p) d -> p oo d", p=P).bitcast(F32R)
            wk_r = w_k.rearrange("(oo p) d -> p oo d", p=P).bitcast(F32R)
            # chunk-interleaved loads: (wq_oo, wk_oo) pairs land every ~3us
            for oo in range(DO):
                nc.scalar.dma_start(wq_sb[:, oo, :], wq_r[:, oo, :])
                nc.scalar.dma_start(wk_sb[:, oo, :], wk_r[:, oo, :])
            with tc.tile_pool(name="mm8", bufs=8, space="PSUM") as mm8:
                for dh in range(2):
                    pm = [
                        mm8.tile([P, 512], F32, tag="mm", name=f"pm{dp}")
                        for dp in range(DO)
                    ]
                    for oo in range(DO):
                        for dp in range(DO):
                            nc.tensor.matmul(
                                pm[dp],
                                wq_sb[:, oo, dp * P : (dp + 1) * P],
                                wk_sb[:, oo, dh * 512 : (dh + 1) * 512],
                                start=(oo == 0),
                                stop=(oo == DO - 1),
                            )
                    for dp in range(DO):
                        nc.vector.tensor_copy(
                            out=mp16[:, dp, dh * 512 : (dh + 1) * 512],
                            in_=pm[dp],
                        )
                    # g = Wk^T bq between the sweeps: wk is fully resident
                    # by now and the round trip finishes well before the
                    # first Z^T drain needs g_col
                    pg = mm8.tile([1, 512], F32, tag="mm", name="pg")
                    for oo in range(DO):
                        nc.tensor.matmul(
                            pg,
                            bq_sb[:, oo : oo + 1],
                            wk_sb[:, oo, dh * 512 : (dh + 1) * 512],
                            start=(oo == 0),
                            stop=(oo == DO - 1),
                        )
                    nc.vector.tensor_copy(
                        out=g_row[:, dh * 512 : (dh + 1) * 512], in_=pg
                    )
        nc.scalar.dma_start(g_d[None, :], g_row)
        nc.scalar.dma_start(g_col, g_d.rearrange("(dd p) -> p dd", p=P))

        # -- Wv^T via PE transposes (cheap: fills the tail of the weight
        #    DMA window), fp16 out straight from PSUM --
        with tc.tile_pool(name="tv_ps", bufs=3, space="PSUM") as tv_psum:
            for oo in range(DO):
                for dq in range(2):
                    tv = tv_psum.tile([P, 4 * P], F32, tag="tv")
                    for k in range(4):
                        dd = 4 * dq + k
                        nc.tensor.transpose(
                            tv[:, k * P : (k + 1) * P],
                            wvbig[:, oo, dd * P : (dd + 1) * P],
                            identity,
                        )
                    nc.vector.tensor_copy(
                        out=wvt[:, 4 * dq : 4 * dq + 4, oo * P : (oo + 1) * P],
                        in_=tv.rearrange("p (k q) -> p k q", k=4),
                    )

        # -- V FIRST (independent of Z^T): its stores release the V
        #    AllGather trigger ~35us earlier, widening the collective
        #    stream's slack against late skew barriers --
        zp = p1.enter_context(tc.tile_pool(name="z_ps", bufs=6, space="PSUM"))
        # -- V local (natural [j, o]; bias bv deferred to epilogue): kept
        #    in SBUF for the local j-blocks AND shipped to DRAM for AG(V) --
        for jj in range(IC):
            pv_h = [
                zp.tile([P, 512], F32, tag="zp", name="pv") for _ in range(2)
            ]
            for dd in range(DO):
                for oh in range(2):
                    nc.tensor.matmul(
                        pv_h[oh],
                        xt16[:, dd, jj * P : (jj + 1) * P],
                        wvt[:, dd, oh * 512 : (oh + 1) * 512],
                        start=(dd == 0),
                        stop=(dd == DO - 1),
                    )
            for oh in range(2):
                # V' = V + bv: softmax rows sum to 1, so attn @ (V + bv)
                # = attn @ V + bv -- the output bias rides the V gather
                # for free and the epilogue needs no bias stage at all
                nc.vector.tensor_tensor(
                    v_sb[:, jj, oh * 512 : (oh + 1) * 512],
                    pv_h[oh],
                    bv_bc[:, oh * 512 : (oh + 1) * 512],
                    ALU.add,
                )
            h, jr = jj // (JBLK // P), jj % (JBLK // P)
            nc.scalar.dma_start(
                v_loc_h[h][jr * P : (jr + 1) * P, :], v_sb[:, jj, :]
            )
            if jr == JBLK // P - 1:
                nc.gpsimd.collective_compute(
                    "AllGather",
                    mybir.AluOpType.bypass,
                    replica_groups=[list(range(NCORES))],
                    ins=[v_loc_h[h].opt()],
                    outs=[v_gh[h].opt()],
                )
                if h == 0:
                    # AG(x^T) half 1: consumed only by the h=1 remote
                    # blocks, which run LAST -- huge deadline slack
                    nc.gpsimd.collective_compute(
                        "AllGather",
                        mybir.AluOpType.bypass,
                        replica_groups=[list(range(NCORES))],
                        ins=[xt_loc_h[1].opt()],
                        outs=[xt_gh[1].opt()],
                    )

        # -- Z^T = M'^T x^T, bias g added on the PSUM drain --
        for dd in range(DO):
            for ih in range(R // JBLK):
                pz = zp.tile([P, JBLK], F32, tag="zp")
                for dp in range(DO):
                    nc.tensor.matmul(
                        pz,
                        mp16[:, dp, dd * P : (dd + 1) * P],
                        xt16[:, dp, ih * JBLK : (ih + 1) * JBLK],
                        start=(dp == 0),
                        stop=(dp == DO - 1),
                    )
                nc.vector.tensor_scalar_add(
                    zt[:, dd, ih * JBLK : (ih + 1) * JBLK],
                    pz,
                    g_col[:, dd : dd + 1],
                )

    # =========================================================
    # Phase 2: streamed attention in E^T layout, rank-rotated
    # =========================================================
    with ExitStack() as p2:
        oa_pool = p2.enter_context(tc.tile_pool(name="oacc", bufs=1))
        outacc = oa_pool.tile([P, IC, D], F32)  # 4 MB

        xtb_pool = p2.enter_context(tc.tile_pool(name="xtb", bufs=3))
        v_pool = p2.enter_context(tc.tile_pool(name="vtb", bufs=3))
        pt_pool = p2.enter_context(tc.tile_pool(name="ptb", bufs=6))
        fin_pool = p2.enter_context(tc.tile_pool(name="fin", bufs=3))
        e_psum = p2.enter_context(tc.tile_pool(name="e_ps", bufs=3, space="PSUM"))
        o_psum = p2.enter_context(tc.tile_pool(name="o_ps", bufs=4, space="PSUM"))
        s_psum = p2.enter_context(tc.tile_pool(name="s_ps", bufs=1, space="PSUM"))

        # rank-rotated slot order: slot 0 is OUR shard (SBUF, no DMA, no
        # dependency on either AllGather); remote slots are read with
        # dynamic-offset DMAs indexed off partition_id
        pid_sy = nc.sync.partition_id()
        rot_sy = [(pid_sy + t) % NCORES for t in range(1, NCORES)]
        pid_gp = nc.gpsimd.partition_id()
        rot_gp = [(pid_gp + t) % NCORES for t in range(1, NCORES)]

        def vtb_prefetch(th):
            """issue the V-rows DMA for remote j-block (t,h) on the GPSIMD
            queue, which is empty in phase 2 (collectives all triggered in
            phase 1): a wait on the V AllGather here never stalls the exp
            (scalar) or xtb (sync) queues feeding the energy pipeline."""
            t, h = th
            vtb = v_pool.tile([P, JBLK // P, D], BF16, tag="vtb", name="vtb")
            nc.gpsimd.dma_start(
                vtb,
                v_gh[h][rot_gp[t - 1]].rearrange("(jj p) o -> p jj o", p=P),
            )
            return vtb

        def e_phase(th):
            """energy matmuls + exp for j-block (t,h); returns ptb.
            Slot t = 0 is local (straight from SBUF)."""
            t, h = th
            if t == 0:
                xsrc = xt16[:, :, h * JBLK : (h + 1) * JBLK]
            else:
                xtb = xtb_pool.tile([P, DO, JBLK], F16, tag="xtb", name="xtb")
                nc.sync.dma_start(
                    xtb, xt_gh[h][rot_sy[t - 1]].rearrange("dd p r -> p dd r")
                )
                xsrc = xtb
            ptb = pt_pool.tile([P, JBLK // P, R], BF16, tag="ptb", name="ptb")
            for jj in range(JBLK // P):
                pe_h = [
                    e_psum.tile([P, JBLK], F32, tag="pe", name="pe")
                    for _ in range(R // JBLK)
                ]
                for dd in range(DO):
                    for ih in range(R // JBLK):
                        nc.tensor.matmul(
                            pe_h[ih],
                            xsrc[:, dd, jj * P : (jj + 1) * P],
                            zt[:, dd, ih * JBLK : (ih + 1) * JBLK],
                            start=(dd == 0),
                            stop=(dd == DO - 1),
                        )
                for ih in range(R // JBLK):
                    nc.scalar.activation(
                        ptb[:, jj, ih * JBLK : (ih + 1) * JBLK],
                        pe_h[ih],
                        AF.Exp,
                        scale=SCALE,
                    )
            return ptb

        def vsrc_of(th, vtb, jj):
            t, h = th
            if t == 0:
                return v_sb[:, h * (JBLK // P) + jj, :]
            return vtb[:, jj, :]

        # ONE psum tile accumulates the exp-sums across ALL 16 j-blocks:
        # PSUM accumulation is free (has_written bits), no per-block DVE
        # read-modify-write, no bank churn at the o-drain
        ps = s_psum.tile([P, 2 * IC], F32, tag="ps", name="ps")

        def o_phase(th, ptb, vtb, first, acc_eng=None):
            """attn@V accumulation + exp-sums for one j-block. acc_eng
            overrides the engine for the outacc accumulation (the
            penultimate block uses GPSIMD so the DVE queue is clear when
            the fused finale needs it)."""
            eng = acc_eng or nc.vector
            for ic in range(IC):
                po_h = [
                    o_psum.tile([P, 512], F32, tag="po", name="po")
                    for _ in range(2)
                ]
                for jj in range(JBLK // P):
                    vs = vsrc_of(th, vtb, jj)
                    for oh in range(2):
                        nc.tensor.matmul(
                            po_h[oh],
                            ptb[:, jj, ic * P : (ic + 1) * P],
                            vs[:, oh * 512 : (oh + 1) * 512],
                            start=(jj == 0),
                            stop=(jj == JBLK // P - 1),
                        )
                    nc.tensor.matmul(
                        ps[:, 2 * ic : 2 * ic + 2],
                        ptb[:, jj, ic * P : (ic + 1) * P],
                        ones,
                        start=(first and ic == 0 and jj == 0),
                        stop=False,
                    )
                for oh in range(2):
                    dst = outacc[:, ic, oh * 512 : (oh + 1) * 512]
                    if first:
                        nc.vector.tensor_copy(out=dst, in_=po_h[oh])
                    else:
                        eng.tensor_tensor(dst, po_h[oh], dst, ALU.add)

        def o_phase_final(parts):
            """merged LAST THREE j-blocks: their attn@V shares one PSUM
            accumulation per (ic, oh), so outacc's last writer is the
            block before them -- the DVE is fully idle when the fused
            normalize + bias + store chain needs it. Exp-sums for all
            merged blocks run first so the reciprocal is ready early."""
            nb_ = len(parts)
            for bi, (th_, ptb_, vtb_) in enumerate(parts):
                for ic in range(IC):
                    for jj in range(JBLK // P):
                        nc.tensor.matmul(
                            ps[:, 2 * ic : 2 * ic + 2],
                            ptb_[:, jj, ic * P : (ic + 1) * P],
                            ones,
                            start=False,
                            stop=(
                                bi == nb_ - 1
                                and ic == IC - 1
                                and jj == JBLK // P - 1
                            ),
                        )
            nc.vector.reciprocal(rs_all, ps)
            for ic in range(IC):
                po_h = [
                    o_psum.tile([P, 512], F32, tag="po", name="po")
                    for _ in range(2)
                ]
                for bi, (th_, ptb_, vtb_) in enumerate(parts):
                    for jj in range(JBLK // P):
                        vs = vsrc_of(th_, vtb_, jj)
                        for oh in range(2):
                            nc.tensor.matmul(
                                po_h[oh],
                                ptb_[:, jj, ic * P : (ic + 1) * P],
                                vs[:, oh * 512 : (oh + 1) * 512],
                                start=(bi == 0 and jj == 0),
                                stop=(bi == nb_ - 1 and jj == JBLK // P - 1),
                            )
                for oh in range(2):
                    # DVE (add outacc) -> ACT (1/s scale) -> DMA out
                    ofin = fin_pool.tile([P, 512], F32, tag="ofin")
                    nc.vector.tensor_tensor(
                        ofin,
                        po_h[oh],
                        outacc[:, ic, oh * 512 : (oh + 1) * 512],
                        ALU.add,
                    )
                    ofin2 = fin_pool.tile([P, 512], F32, tag="ofin2")
                    nc.scalar.activation(
                        ofin2, ofin, AF.Copy,
                        scale=rs_all[:, 2 * ic : 2 * ic + 1],
                    )
                    nc.sync.dma_start(
                        out_loc[ic * P : (ic + 1) * P, oh * 512 : (oh + 1) * 512],
                        ofin2,
                    )

        # Block sequence: the two LOCAL blocks first (they need neither
        # AllGather), then ALL h=0 remotes, then ALL h=1 remotes -- so
        # only AG1a/AG2a are timing-critical; the h=1 gathers get ~200us
        # of extra deadline slack. Emission: an eager run of EAGER_E
        # remote e-phases builds runway so the V gather hides behind
        # energy work even when the skew barrier lands late; after that
        # e/o interleave 1:1 and the o's drain at the end.
        seq = [(0, 0), (0, 1)]
        seq += [(t, 0) for t in range(1, NCORES)]
        seq += [(t, 1) for t in range(1, NCORES)]

        NFIN = 3  # blocks merged into the finale
        order = [("e", 0), ("e", 1), ("o", 0), ("o", 1)]
        order += [("e", b) for b in range(2, 2 + EAGER_E)]
        nb = 2 + EAGER_E
        for b in range(2, NJB - NFIN):
            order.append(("o", b))
            if nb < NJB:
                order.append(("e", nb))
                nb += 1
        while nb < NJB:
            order.append(("e", nb))
            nb += 1
        order.append(("fin", NJB - NFIN))

        ptbs, vtbs = {}, {}
        for kind, b in order:
            if kind == "e":
                ptbs[b] = e_phase(seq[b])
            elif kind == "o":
                # prefetch the NEXT remote o-block's V rows one full
                # o-phase (~18us) ahead of its consumption
                if 2 <= b + 1 < NJB:
                    vtbs[b + 1] = vtb_prefetch(seq[b + 1])
                o_phase(seq[b], ptbs.pop(b), vtbs.pop(b, None), b == 0)
            else:
                # remaining vtb prefetches lead their use by the upfront
                # sum-matmul batch (~5us)
                for bf in range(b + 1, NJB):
                    vtbs[bf] = vtb_prefetch(seq[bf])
                o_phase_final(
                    [(seq[bf], ptbs.pop(bf), vtbs.pop(bf, None))
                     for bf in range(b, NJB)]
                )

    outer.close()


_NC_CACHE = None


def _get_program():
    global _NC_CACHE
    if _NC_CACHE is None:
        _NC_CACHE = build_program()
    return _NC_CACHE


def _run(inputs, trace=False):
    nc = _get_program()
    x = np.ascontiguousarray(np.asarray(inputs["x"], dtype=np.float32))
    common = {
        k: np.ascontiguousarray(np.asarray(inputs[k], dtype=np.float32))
        for k in ("Wq", "Wk", "Wv", "bq", "bv")
    }
    in_maps = [
        {"x_loc": np.ascontiguousarray(x[c * R : (c + 1) * R]), **common}
        for c in range(NCORES)
    ]
    res = run_bass_kernel_spmd(
        nc, in_maps, core_ids=list(range(NCORES)), trace=trace
    )
    out = np.concatenate([res.results[c]["out_loc"] for c in range(NCORES)], axis=0)
    return out.reshape(B, D, 1).astype(np.float32), res


def kernel(**inputs):
    out, _ = _run(inputs, trace=False)
    return out
